# revision 9
# baseline (speedup 1.0000x reference)
"""Trainium2 Bass kernel for a 4-round GCN (BlockGNN) over 8 NeuronCores.

Strategy (self-contained; shapes/sharding hardcoded for the staged problem):
  - Nodes are split into 128-row blocks; each of the 8 cores owns NBC blocks
    (dst-sharding). Edges (incl. self-loops) are grouped by dst block on host;
    GCN normalization coefficients are folded into per-chunk selection
    matrices S (S[e, m] = (dst_local[e]==m) * norm[e]) built on-chip with one
    fused DVE tensor_scalar op per 128-edge chunk.
  - Aggregation out = sum_e norm[e] * z[src[e]] is computed as TensorEngine
    matmuls: psum[128 nodes, H] += S_chunk.T @ gathered_chunk, with z rows
    fetched by GPSIMD dma_gather (int16 indices -> lo/hi split tables).
  - Per layer, transformed features z = (h + res) @ W are computed node-
    sharded and AllGathered (halo exchange) into a replicated z table.
  - Graph mean-pooling is a per-block one-hot matmul accumulated in PSUM,
    AllReduced across cores; the small head (gm @ Wlin) runs replicated.
"""
import os
import hashlib
import math
import numpy as np

import concourse.bass as bass
import concourse.bacc as bacc
import concourse.tile as tile
import concourse.mybir as mybir
from concourse.bass_utils import run_bass_kernel_spmd

NCORES = 8
P = 128
LO_SPLIT = 32768          # int16 gather index limit
RES_GRP = 1024            # res gather group size (indices per dma_gather)
ZG = 8                    # blocks per z-write granule
XG = 8                    # blocks per xT load granule (layer 0)

_CACHE = {}
LAST = {"exec_ns": None, "results": None}


def _wrap16(flat, pad_val):
    """idx list -> [128, len/16] int16 (pos i at partition i%16, col i//16;
    replicated to 128 partitions)."""
    n = len(flat)
    assert n % 16 == 0
    w = np.asarray(flat, dtype=np.int16).reshape(n // 16, 16).T  # [16, n/16]
    return np.tile(w, (8, 1))


def _preprocess(x, edge_index, batch, graph_hidden, W0, b0, Ws, bs, Wlin, blin,
                dt_np):
    N, F = x.shape
    H = W0.shape[1]
    G, GH = graph_hidden.shape
    L = Ws.shape[0]
    C = Wlin.shape[1]
    assert F == P, "input feature dim must be 128"
    assert H % P == 0 and H <= 512
    assert GH == H and G <= P
    HK = H // P

    NBC = math.ceil(N / (NCORES * P))     # blocks per core
    NPC = NBC * P                          # padded nodes per core
    NTOT = NCORES * NPC
    NBT = NCORES * NBC

    src = np.concatenate([edge_index[0], np.arange(N)]).astype(np.int64)
    dst = np.concatenate([edge_index[1], np.arange(N)]).astype(np.int64)
    deg = np.bincount(dst, minlength=N).astype(np.float64)
    inv = 1.0 / np.sqrt(np.maximum(deg, 1.0))
    norm = (inv[src] * inv[dst]).astype(np.float32)

    # node -> slot permutation. "balance" snake-deals nodes into blocks by
    # lo-indegree so every block sees a near-equal lo/hi edge load (smaller
    # CL/CH quotas => less chunk padding). Nodes keep their table region
    # (old id < LO_SPLIT  <=>  new id < LO_SPLIT) so int16 index splitting
    # is unaffected.
    balance = os.environ.get("BASSGNN_PERM", "balance") == "balance"
    LOR = min(LO_SPLIT, NTOT)
    n_lo_real = min(N, LOR)
    if balance:
        is_hi_old = src >= LO_SPLIT
        indeg_lo = np.bincount(dst[~is_hi_old], minlength=N).astype(np.int64)
        indeg_hi = np.bincount(dst[is_hi_old], minlength=N).astype(np.int64)

        def _snake(ids, nblk):
            w = np.where(ids >= 0, indeg_lo[np.maximum(ids, 0)], -1)
            order = np.argsort(-w, kind="stable")
            grid = ids[order].reshape(P, nblk)
            grid[1::2] = grid[1::2, ::-1]
            return grid.T.reshape(-1)  # block-major slots

        lo_ids = np.concatenate(
            [np.arange(n_lo_real), np.full(LOR - n_lo_real, -1)])
        parts = [_snake(lo_ids, LOR // P)]
        if NTOT > LOR:
            hi_ids = np.concatenate(
                [np.arange(n_lo_real, N), np.full(NTOT - LOR - (N - n_lo_real), -1)])
            parts.append(_snake(hi_ids, (NTOT - LOR) // P))
        old_of_new = np.concatenate(parts)
    else:
        old_of_new = np.concatenate([np.arange(N), np.full(NTOT - N, -1)])
    slot_mask = old_of_new >= 0
    new_of_old = np.zeros(N, np.int64)
    new_of_old[old_of_new[slot_mask]] = np.nonzero(slot_mask)[0]
    src = new_of_old[src]
    dst = new_of_old[dst]
    x_pad = np.zeros((NTOT, P), np.float32)
    x_pad[slot_mask] = x[old_of_new[slot_mask]]
    batch_new = np.zeros(NTOT, np.int64)
    batch_new[slot_mask] = batch[old_of_new[slot_mask]]
    pool_new = np.full(NTOT, -1.0, np.float32)
    pool_new[slot_mask] = batch[old_of_new[slot_mask]]

    blk = dst // P
    is_hi = src >= LO_SPLIT
    order = np.lexsort((is_hi, blk))
    blk_s = blk[order]
    hi_s = is_hi[order]
    src_s = src[order]
    dst_s = dst[order]
    nrm_s = norm[order]
    bstart = np.searchsorted(blk_s, np.arange(NBT))
    bend = np.searchsorted(blk_s, np.arange(NBT), side="right")
    n_lo_b = np.zeros(NBT, np.int64)
    n_hi_b = np.zeros(NBT, np.int64)
    for b in range(NBT):
        seg_hi = hi_s[bstart[b]:bend[b]]
        n_hi_b[b] = seg_hi.sum()
        n_lo_b[b] = len(seg_hi) - n_hi_b[b]
    CL = max(1, math.ceil(n_lo_b.max() / P))
    any_hi = (NTOT > LO_SPLIT) and n_hi_b.max() > 0
    CH = max(1, math.ceil(n_hi_b.max() / P)) if any_hi else 0
    CPB = CL + CH

    idx_lo = np.full((NCORES, P, NBC * CL * 8), -1, np.int16)
    idx_hi = np.full((NCORES, P, NBC * CH * 8), -1, np.int16) if CH else None
    dstl = np.full((NCORES, P, NBC * CPB), -1.0, np.float32)
    nrmv = np.zeros((NCORES, P, NBC * CPB), np.float32)
    cnts = np.zeros((NCORES, 1, NBC * 2), np.int32)

    for b in range(NBT):
        k, lb = divmod(b, NBC)
        s0, s1 = bstart[b], bend[b]
        nl = int(n_lo_b[b])
        nh = int(n_hi_b[b])
        # lo side
        fl = np.full(CL * P, -1, np.int64)
        dl = np.full(CL * P, -1.0, np.float32)
        vl = np.zeros(CL * P, np.float32)
        if nl:
            fl[:nl] = src_s[s0:s0 + nl]
            dl[:nl] = dst_s[s0:s0 + nl] - b * P
            vl[:nl] = nrm_s[s0:s0 + nl]
        else:
            fl[0] = 0  # dummy valid idx (norm stays 0)
            nl = 1
        idx_lo[k][:, lb * CL * 8:(lb + 1) * CL * 8] = _wrap16(fl, -1)
        dstl[k][:, lb * CPB:lb * CPB + CL] = dl.reshape(CL, P).T
        nrmv[k][:, lb * CPB:lb * CPB + CL] = vl.reshape(CL, P).T
        cnts[k][0, 2 * lb] = nl
        # hi side
        if CH:
            fh = np.full(CH * P, -1, np.int64)
            dh = np.full(CH * P, -1.0, np.float32)
            vh = np.zeros(CH * P, np.float32)
            if nh:
                fh[:nh] = src_s[s1 - nh:s1] - LO_SPLIT
                dh[:nh] = dst_s[s1 - nh:s1] - b * P
                vh[:nh] = nrm_s[s1 - nh:s1]
            else:
                fh[0] = 0
                nh = 1
            idx_hi[k][:, lb * CH * 8:(lb + 1) * CH * 8] = _wrap16(fh, -1)
            dstl[k][:, lb * CPB + CL:(lb + 1) * CPB] = dh.reshape(CH, P).T
            nrmv[k][:, lb * CPB + CL:(lb + 1) * CPB] = vh.reshape(CH, P).T
            cnts[k][0, 2 * lb + 1] = nh

    # res gather indices (graph id per node) and pooling one-hot scalars
    batch_pad = batch_new
    pool_v = pool_new
    res_groups = []
    off = 0
    while off < NPC:
        g = min(RES_GRP, NPC - off)
        res_groups.append(g)
        off += g
    idx_res = np.zeros((NCORES, P, NPC // 16), np.int16)
    batchv = np.zeros((NCORES, P, NBC), np.float32)
    for k in range(NCORES):
        off = 0
        col = 0
        for g in res_groups:
            seg = batch_pad[k * NPC + off:k * NPC + off + g]
            idx_res[k][:, col:col + g // 16] = _wrap16(seg, 0)
            off += g
            col += g // 16
        batchv[k] = pool_v[k * NPC:(k + 1) * NPC].reshape(NBC, P).T

    counts = np.bincount(batch, minlength=G).astype(np.float32)
    inv_cnt = np.zeros((P, 1), np.float32)
    inv_cnt[:G, 0] = 1.0 / np.maximum(counts, 1.0)

    xT = np.ascontiguousarray(x_pad.T).astype(dt_np)

    gh_f = np.zeros((P, H), np.float32)
    gh_f[:G] = graph_hidden
    iota = np.tile(np.arange(P, dtype=np.float32)[None, :], (P, 1))
    ident = np.eye(P, dtype=np.float32)

    ws_np = np.zeros((L, HK, P, H), np.float32)
    for l in range(L):
        for kk in range(HK):
            ws_np[l, kk] = Ws[l][kk * P:(kk + 1) * P, :]
    wlin_np = np.zeros((HK, P, C), np.float32)
    for kk in range(HK):
        wlin_np[kk] = Wlin[kk * P:(kk + 1) * P, :]

    has_bias = bool(np.any(b0 != 0) or np.any(bs != 0) or np.any(blin != 0))
    bias_np = np.zeros((L + 1, P, H), np.float32)
    bias_np[0] = np.tile(b0[None, :], (P, 1))
    for l in range(L):
        bias_np[l + 1] = np.tile(bs[l][None, :], (P, 1))
    blin_np = np.tile(blin[None, :], (P, 1)).astype(np.float32)

    if os.environ.get("BASSGNN_VERBOSE", "1") == "1":
        print(f"[kernel] CL={CL} CH={CH} CPB={CPB} NBC={NBC} NTOT={NTOT}")
    cfg = dict(N=N, H=H, G=G, L=L, C=C, HK=HK, NBC=NBC, NPC=NPC, NTOT=NTOT,
               CL=CL, CH=CH, CPB=CPB, res_groups=tuple(res_groups),
               has_bias=has_bias)

    in_maps = []
    for k in range(NCORES):
        m = {
            "xT": xT,
            "w0": W0.astype(dt_np),
            "ws": ws_np.astype(dt_np),
            "wlin": wlin_np.astype(np.float32),
            "gh": gh_f,
            "gh_dt": gh_f.astype(dt_np),
            "iota": iota.astype(dt_np),
            "ident": ident.astype(dt_np),
            "ident32": ident,
            "inv_cnt": inv_cnt,
            "idx_lo": idx_lo[k],
            "dstl": dstl[k],
            "nrmv": nrmv[k],
            "cnts": cnts[k],
            "idx_res": idx_res[k],
            "batchv": batchv[k],
        }
        if CH:
            m["idx_hi"] = idx_hi[k]
        if has_bias:
            m["biasb"] = bias_np
            m["blinb"] = blin_np
        in_maps.append(m)
    return cfg, in_maps


def _build(cfg, dt):
    H, G, L, C, HK = cfg["H"], cfg["G"], cfg["L"], cfg["C"], cfg["HK"]
    NBC, NPC, NTOT = cfg["NBC"], cfg["NPC"], cfg["NTOT"]
    CL, CH, CPB = cfg["CL"], cfg["CH"], cfg["CPB"]
    res_groups = cfg["res_groups"]
    has_bias = cfg["has_bias"]
    NBT = NTOT // P
    f32 = mybir.dt.float32
    AF = mybir.ActivationFunctionType

    nc = bacc.Bacc("TRN2", target_bir_lowering=False, debug=False,
                   num_devices=NCORES, enable_asserts=False)

    xT_d = nc.dram_tensor("xT", [P, NTOT], dt, kind="ExternalInput")
    w0_d = nc.dram_tensor("w0", [P, H], dt, kind="ExternalInput")
    ws_d = nc.dram_tensor("ws", [L, HK, P, H], dt, kind="ExternalInput")
    wlin_d = nc.dram_tensor("wlin", [HK, P, C], f32, kind="ExternalInput")
    gh_d = nc.dram_tensor("gh", [P, H], f32, kind="ExternalInput")
    ghdt_d = nc.dram_tensor("gh_dt", [P, H], dt, kind="ExternalInput")
    iota_d = nc.dram_tensor("iota", [P, P], dt, kind="ExternalInput")
    ident_d = nc.dram_tensor("ident", [P, P], dt, kind="ExternalInput")
    ident32_d = nc.dram_tensor("ident32", [P, P], f32, kind="ExternalInput")
    invc_d = nc.dram_tensor("inv_cnt", [P, 1], f32, kind="ExternalInput")
    idxlo_d = nc.dram_tensor("idx_lo", [P, NBC * CL * 8], mybir.dt.int16, kind="ExternalInput")
    idxhi_d = nc.dram_tensor("idx_hi", [P, NBC * CH * 8], mybir.dt.int16, kind="ExternalInput") if CH else None
    dstl_d = nc.dram_tensor("dstl", [P, NBC * CPB], f32, kind="ExternalInput")
    nrmv_d = nc.dram_tensor("nrmv", [P, NBC * CPB], f32, kind="ExternalInput")
    cnts_d = nc.dram_tensor("cnts", [1, NBC * 2], mybir.dt.int32, kind="ExternalInput")
    idxres_d = nc.dram_tensor("idx_res", [P, NPC // 16], mybir.dt.int16, kind="ExternalInput")
    batchv_d = nc.dram_tensor("batchv", [P, NBC], f32, kind="ExternalInput")
    if has_bias:
        biasb_d = nc.dram_tensor("biasb", [L + 1, P, H], f32, kind="ExternalInput")
        blinb_d = nc.dram_tensor("blinb", [P, C], f32, kind="ExternalInput")

    y_out = nc.dram_tensor("y_out", [P, C], f32, kind="ExternalOutput")
    gm_out = nc.dram_tensor("gm_out", [P, H], f32, kind="ExternalOutput")

    with tile.TileContext(nc) as tc:
        with tc.tile_pool(name="const", bufs=1) as cpool, \
             tc.tile_pool(name="dram", bufs=1, space="DRAM") as dram, \
             tc.tile_pool(name="psum", bufs=1, space="PSUM") as pp, \
             tc.tile_pool(name="sacc", bufs=8) as spool, \
             tc.tile_pool(name="work", bufs=1) as wpool, \
             tc.tile_pool(name="gran", bufs=2) as gpool:

            # ---- resident constants / metadata ----
            w0_sb = cpool.tile([P, H], dt)
            nc.sync.dma_start(w0_sb[:], w0_d[:])
            ws_sb = cpool.tile([P, L, HK, H], dt)
            nc.sync.dma_start(ws_sb[:], ws_d.ap().rearrange("l k p h -> p l k h"))
            wlin_sb = cpool.tile([P, HK, C], f32)
            nc.sync.dma_start(wlin_sb[:], wlin_d.ap().rearrange("k p c -> p k c"))
            gh_sb = cpool.tile([P, H], f32)
            nc.sync.dma_start(gh_sb[:], gh_d[:])
            iota_sb = cpool.tile([P, P], dt)
            nc.sync.dma_start(iota_sb[:], iota_d[:])
            ident_sb = cpool.tile([P, P], dt)
            nc.sync.dma_start(ident_sb[:], ident_d[:])
            ident32_sb = cpool.tile([P, P], f32)
            nc.sync.dma_start(ident32_sb[:], ident32_d[:])
            invc_sb = cpool.tile([P, 1], f32)
            nc.sync.dma_start(invc_sb[:], invc_d[:])
            idxlo_sb = cpool.tile([P, NBC * CL * 8], mybir.dt.int16)
            nc.sync.dma_start(idxlo_sb[:], idxlo_d[:])
            if CH:
                idxhi_sb = cpool.tile([P, NBC * CH * 8], mybir.dt.int16)
                nc.sync.dma_start(idxhi_sb[:], idxhi_d[:])
            dstl_sb = cpool.tile([P, NBC * CPB], f32)
            nc.sync.dma_start(dstl_sb[:], dstl_d[:])
            nrmv_sb = cpool.tile([P, NBC * CPB], f32)
            nc.sync.dma_start(nrmv_sb[:], nrmv_d[:])
            cnts_sb = cpool.tile([P, NBC * 2], mybir.dt.int32)
            nc.sync.dma_start(cnts_sb[0:1, :], cnts_d[:])
            idxres_sb = cpool.tile([P, NPC // 16], mybir.dt.int16)
            nc.sync.dma_start(idxres_sb[:], idxres_d[:])
            batchv_sb = cpool.tile([P, NBC], f32)
            nc.sync.dma_start(batchv_sb[:], batchv_d[:])
            if has_bias:
                biasb_sb = cpool.tile([P, L + 1, H], f32)
                nc.sync.dma_start(biasb_sb[:], biasb_d.ap().rearrange("l p h -> p l h"))
                blinb_sb = cpool.tile([P, C], f32)
                nc.sync.dma_start(blinb_sb[:], blinb_d[:])

            # ---- DRAM state ----
            zt = dram.tile([NTOT, H], dt)            # replicated z table
            zin = dram.tile([NPC, H], dt)            # this core's z slice
            res_dram = dram.tile([NPC, H], dt)       # graph_hidden[batch] rows
            ar_in = dram.tile([P, H], f32)
            ar_out = dram.tile([P, H], f32)

            # ---- persistent SBUF state ----
            h_sb = wpool.tile([P, NBC, H], dt)       # per-block activations
            msgA = wpool.tile([P, CPB, H], dt)
            msgB = wpool.tile([P, CPB, H], dt)
            nc.vector.memset(msgA[:], 0.0)
            nc.vector.memset(msgB[:], 0.0)

            # ---- res rows: gather graph_hidden[batch] into res_dram ----
            off = 0
            col = 0
            for g in res_groups:
                stg = gpool.tile([P, RES_GRP // P, H], dt, tag="resg")
                nc.gpsimd.dma_gather(
                    stg[:, :g // P, :], ghdt_d.ap(),
                    idxres_sb[:, col:col + g // 16],
                    num_idxs=g, num_idxs_reg=g, elem_size=H,
                    single_packet=False)
                nc.sync.dma_start(
                    res_dram[off:off + g, :].rearrange("(g p) h -> p g h", p=P),
                    stg[:, :g // P, :])
                off += g
                col += g // 16

            # ---- layer 0: z0 = x @ W0, computed for ALL blocks (replicated) ----
            for g0 in range(0, NBT, XG):
                gsz = min(XG, NBT - g0)
                xt_g = gpool.tile([P, XG * P], dt, tag="xtg")
                nc.sync.dma_start(xt_g[:, :gsz * P], xT_d[:, g0 * P:(g0 + gsz) * P])
                zg = gpool.tile([P, XG, H], dt, tag="zg0")
                for j in range(gsz):
                    acc = pp.tile([P, H], f32, tag="acc", bufs=3)
                    nc.tensor.matmul(acc[:], xt_g[:, j * P:(j + 1) * P], w0_sb[:],
                                     start=True, stop=True)
                    if j % 2 == 0:
                        nc.scalar.activation(zg[:, j, :], acc[:], AF.Copy)
                    else:
                        nc.vector.tensor_copy(zg[:, j, :], acc[:])
                nc.sync.dma_start(
                    zt[g0 * P:(g0 + gsz) * P, :].rearrange("(g p) h -> p g h", p=P),
                    zg[:, :gsz, :])

            zt_lo = zt[0:min(LO_SPLIT, NTOT), :]
            zt_hi = zt[LO_SPLIT:NTOT, :] if CH else None

            # ---- aggregation rounds ----
            for r in range(L + 1):
                if r > 0:
                    # phase A: z = (h + res) @ Ws[r-1], node-sharded + AllGather

                    for b in range(NBC):
                        if b % ZG == 0:
                            gsz = min(ZG, NBC - b)
                            res_g = gpool.tile([P, ZG, H], dt, tag="resg")
                            nc.sync.dma_start(
                                res_g[:, :gsz, :],
                                res_dram[b * P:(b + gsz) * P, :].rearrange(
                                    "(g p) h -> p g h", p=P))
                            zg2 = gpool.tile([P, ZG, H], dt, tag="zg2")
                        hpr = gpool.tile([P, H], dt, tag="hpr", bufs=3)
                        nc.vector.tensor_tensor(
                            hpr[:], h_sb[:, b, :], res_g[:, b % ZG, :],
                            mybir.AluOpType.add)
                        hprT = gpool.tile([P, HK * P], dt, tag="hprT", bufs=3)
                        for kk in range(HK):
                            pt = pp.tile([P, P], dt, tag="tr", bufs=2)
                            nc.tensor.transpose(
                                pt[:], hpr[:, kk * P:(kk + 1) * P], ident_sb[:])
                            nc.vector.tensor_copy(hprT[:, kk * P:(kk + 1) * P], pt[:])
                        accz = pp.tile([P, H], f32, tag="acc", bufs=3)
                        for kk in range(HK):
                            nc.tensor.matmul(
                                accz[:], hprT[:, kk * P:(kk + 1) * P],
                                ws_sb[:, r - 1, kk, :],
                                start=(kk == 0), stop=(kk == HK - 1))
                        nc.scalar.activation(zg2[:, b % ZG, :], accz[:], AF.Copy)
                        if b % ZG == ZG - 1 or b == NBC - 1:
                            b0g = (b // ZG) * ZG
                            gsz = b - b0g + 1
                            nc.sync.dma_start(
                                zin[b0g * P:(b0g + gsz) * P, :].rearrange(
                                    "(g p) h -> p g h", p=P),
                                zg2[:, :gsz, :])
                    nc.gpsimd.collective_compute(
                        "AllGather", mybir.AluOpType.bypass,
                        replica_groups=[list(range(NCORES))],
                        ins=[zin[:].opt()], outs=[zt[:].opt()])

                # phase B: gather + segment-sum matmuls per owned block
                for b in range(NBC):
                    msg = msgA if b % 2 == 0 else msgB
                    rlo = nc.gpsimd.alloc_register(f"rlo_{r}_{b}")
                    nc.gpsimd.reg_load(rlo, cnts_sb[0:1, 2 * b:2 * b + 1])
                    nc.gpsimd.dma_gather(
                        msg[:, 0:CL, :], zt_lo,
                        idxlo_sb[:, b * CL * 8:(b + 1) * CL * 8],
                        num_idxs=CL * P, num_idxs_reg=rlo, elem_size=H,
                        single_packet=False)
                    if CH:
                        rhi = nc.gpsimd.alloc_register(f"rhi_{r}_{b}")
                        nc.gpsimd.reg_load(rhi, cnts_sb[0:1, 2 * b + 1:2 * b + 2])
                        nc.gpsimd.dma_gather(
                            msg[:, CL:CPB, :], zt_hi,
                            idxhi_sb[:, b * CH * 8:(b + 1) * CH * 8],
                            num_idxs=CH * P, num_idxs_reg=rhi, elem_size=H,
                            single_packet=False)
                    acch = pp.tile([P, H], f32, tag="acc", bufs=3)
                    for c in range(CPB):
                        S = spool.tile([P, P], dt, tag="S")
                        nc.vector.tensor_scalar(
                            S[:], iota_sb[:],
                            dstl_sb[:, b * CPB + c:b * CPB + c + 1],
                            nrmv_sb[:, b * CPB + c:b * CPB + c + 1],
                            mybir.AluOpType.is_equal, mybir.AluOpType.mult)
                        nc.tensor.matmul(acch[:], S[:], msg[:, c, :],
                                         start=(c == 0), stop=(c == CPB - 1))
                    if has_bias:
                        tmpb = gpool.tile([P, H], f32, tag="tmpb", bufs=2)
                        nc.vector.tensor_tensor(
                            tmpb[:], acch[:], biasb_sb[:, r, :],
                            mybir.AluOpType.add)
                        nc.scalar.activation(h_sb[:, b, :], tmpb[:],
                                             AF.Relu if r > 0 else AF.Copy)
                    else:
                        nc.scalar.activation(h_sb[:, b, :], acch[:],
                                             AF.Relu if r > 0 else AF.Copy)

            # ---- graph mean pooling ----
            accg = pp.tile([P, H], f32, tag="gm")
            for b in range(NBC):
                Sp = spool.tile([P, P], dt, tag="S")
                nc.vector.tensor_scalar(
                    Sp[:], iota_sb[:], batchv_sb[:, b:b + 1], None,
                    mybir.AluOpType.is_equal)
                nc.tensor.matmul(accg[:], Sp[:], h_sb[:, b, :],
                                 start=(b == 0), stop=(b == NBC - 1))
            gm_part = wpool.tile([P, H], f32)
            nc.vector.tensor_copy(gm_part[:], accg[:])
            nc.sync.dma_start(ar_in[:], gm_part[:])
            nc.gpsimd.collective_compute(
                "AllReduce", mybir.AluOpType.add,
                replica_groups=[list(range(NCORES))],
                ins=[ar_in[:].opt()], outs=[ar_out[:].opt()])
            gm_sum = wpool.tile([P, H], f32)
            nc.sync.dma_start(gm_sum[:], ar_out[:])
            gm_sb = wpool.tile([P, H], f32)
            nc.vector.tensor_scalar(gm_sb[:], gm_sum[:], invc_sb[:, 0:1], None,
                                    mybir.AluOpType.mult)
            nc.vector.tensor_tensor(gm_sb[:], gm_sb[:], gh_sb[:],
                                    mybir.AluOpType.add)
            nc.sync.dma_start(gm_out[:], gm_sb[:])

            # ---- head: y = gm @ Wlin (+ blin) ----
            gmT = wpool.tile([P, HK * P], f32)
            for kk in range(HK):
                pt = pp.tile([P, P], f32, tag="tr", bufs=2)
                nc.tensor.transpose(pt[:], gm_sb[:, kk * P:(kk + 1) * P],
                                    ident32_sb[:])
                nc.vector.tensor_copy(gmT[:, kk * P:(kk + 1) * P], pt[:])
            accy = pp.tile([P, C], f32, tag="y")
            for kk in range(HK):
                nc.tensor.matmul(accy[:], gmT[:, kk * P:(kk + 1) * P],
                                 wlin_sb[:, kk, :],
                                 start=(kk == 0), stop=(kk == HK - 1))
            y_sb = wpool.tile([P, C], f32)
            if has_bias:
                nc.vector.tensor_tensor(y_sb[:], accy[:], blinb_sb[:],
                                        mybir.AluOpType.add)
            else:
                nc.vector.tensor_copy(y_sb[:], accy[:])
            nc.sync.dma_start(y_out[:], y_sb[:])

    nc.finalize()
    return nc


def kernel(x, edge_index, batch, graph_hidden, W0, b0, Ws, bs, Wlin, blin):
    x = np.asarray(x, np.float32)
    edge_index = np.asarray(edge_index)
    batch = np.asarray(batch)
    graph_hidden = np.asarray(graph_hidden, np.float32)
    W0 = np.asarray(W0, np.float32)
    b0 = np.asarray(b0, np.float32)
    Ws = np.asarray(Ws, np.float32)
    bs = np.asarray(bs, np.float32)
    Wlin = np.asarray(Wlin, np.float32)
    blin = np.asarray(blin, np.float32)

    use_bf16 = os.environ.get("BASSGNN_DT", "f32") == "bf16"
    dt = mybir.dt.bfloat16 if use_bf16 else mybir.dt.float32
    dt_np = mybir.dt.np(dt)

    cfg, in_maps = _preprocess(x, edge_index, batch, graph_hidden,
                               W0, b0, Ws, bs, Wlin, blin, dt_np)
    key = (x.shape, edge_index.shape, cfg["CL"], cfg["CH"], cfg["has_bias"],
           str(dt), hashlib.sha1(np.ascontiguousarray(edge_index).tobytes()).hexdigest())
    if key not in _CACHE:
        _CACHE[key] = _build(cfg, dt)
    nc = _CACHE[key]

    trace = os.environ.get("BASSGNN_TRACE", "0") == "1"
    res = run_bass_kernel_spmd(nc, in_maps, core_ids=list(range(NCORES)),
                               trace=trace)
    LAST["exec_ns"] = res.exec_time_ns
    LAST["results"] = res
    G = cfg["G"]
    y = np.asarray(res.results[0]["y_out"][:G], np.float32)
    gm = np.asarray(res.results[0]["gm_out"][:G], np.float32)
    return (y, gm)


# revision 11
# speedup vs baseline: 1.5781x; 1.5781x over previous
"""Trainium2 Bass kernel for a 4-round GCN (BlockGNN) over 8 NeuronCores.

Strategy (self-contained; shapes/sharding hardcoded for the staged problem):
  - Nodes are split into 128-row blocks; each of the 8 cores owns NBC blocks
    (dst-sharding). Edges (incl. self-loops) are grouped by dst block on host;
    GCN normalization coefficients are folded into per-chunk selection
    matrices S (S[e, m] = (dst_local[e]==m) * norm[e]) built on-chip with one
    fused DVE tensor_scalar op per 128-edge chunk.
  - Aggregation out = sum_e norm[e] * z[src[e]] is computed as TensorEngine
    matmuls: psum[128 nodes, H] += S_chunk.T @ gathered_chunk, with z rows
    fetched by GPSIMD dma_gather (int16 indices -> lo/hi split tables).
  - Per layer, transformed features z = (h + res) @ W are computed node-
    sharded and AllGathered (halo exchange) into a replicated z table.
  - Graph mean-pooling is a per-block one-hot matmul accumulated in PSUM,
    AllReduced across cores; the small head (gm @ Wlin) runs replicated.
"""
import os
import hashlib
import math
import numpy as np

import concourse.bass as bass
import concourse.bacc as bacc
import concourse.tile as tile
import concourse.mybir as mybir
from concourse.bass_utils import run_bass_kernel_spmd

NCORES = 8
P = 128
LO_SPLIT = 32768          # int16 gather index limit
RES_GRP = 1024            # res gather group size (indices per dma_gather)
ZG = 8                    # blocks per z-write granule
XG = 8                    # blocks per xT load granule (layer 0)

_CACHE = {}
LAST = {"exec_ns": None, "results": None}


def _wrap16(flat, pad_val):
    """idx list -> [128, len/16] int16 (pos i at partition i%16, col i//16;
    replicated to 128 partitions)."""
    n = len(flat)
    assert n % 16 == 0
    w = np.asarray(flat, dtype=np.int16).reshape(n // 16, 16).T  # [16, n/16]
    return np.tile(w, (8, 1))


def _preprocess(x, edge_index, batch, graph_hidden, W0, b0, Ws, bs, Wlin, blin,
                dt_np):
    N, F = x.shape
    H = W0.shape[1]
    G, GH = graph_hidden.shape
    L = Ws.shape[0]
    C = Wlin.shape[1]
    assert F == P, "input feature dim must be 128"
    assert H % P == 0 and H <= 512
    assert GH == H and G <= P
    HK = H // P

    NBC = math.ceil(N / (NCORES * P))     # blocks per core
    NPC = NBC * P                          # padded nodes per core
    NTOT = NCORES * NPC
    NBT = NCORES * NBC

    src = np.concatenate([edge_index[0], np.arange(N)]).astype(np.int64)
    dst = np.concatenate([edge_index[1], np.arange(N)]).astype(np.int64)
    deg = np.bincount(dst, minlength=N).astype(np.float64)
    inv = 1.0 / np.sqrt(np.maximum(deg, 1.0))
    norm = (inv[src] * inv[dst]).astype(np.float32)

    # node -> slot permutation. "balance" snake-deals nodes into blocks by
    # lo-indegree so every block sees a near-equal lo/hi edge load (smaller
    # CL/CH quotas => less chunk padding). Nodes keep their table region
    # (old id < LO_SPLIT  <=>  new id < LO_SPLIT) so int16 index splitting
    # is unaffected.
    balance = os.environ.get("BASSGNN_PERM", "balance") == "balance"
    LOR = min(LO_SPLIT, NTOT)
    n_lo_real = min(N, LOR)
    if balance:
        is_hi_old = src >= LO_SPLIT
        indeg_lo = np.bincount(dst[~is_hi_old], minlength=N).astype(np.int64)
        indeg_hi = np.bincount(dst[is_hi_old], minlength=N).astype(np.int64)

        def _snake(ids, nblk):
            w = np.where(ids >= 0, indeg_lo[np.maximum(ids, 0)], -1)
            order = np.argsort(-w, kind="stable")
            grid = ids[order].reshape(P, nblk)
            grid[1::2] = grid[1::2, ::-1]
            return grid.T.reshape(-1)  # block-major slots

        lo_ids = np.concatenate(
            [np.arange(n_lo_real), np.full(LOR - n_lo_real, -1)])
        parts = [_snake(lo_ids, LOR // P)]
        if NTOT > LOR:
            hi_ids = np.concatenate(
                [np.arange(n_lo_real, N), np.full(NTOT - LOR - (N - n_lo_real), -1)])
            parts.append(_snake(hi_ids, (NTOT - LOR) // P))
        old_of_new = np.concatenate(parts)
    else:
        old_of_new = np.concatenate([np.arange(N), np.full(NTOT - N, -1)])
    slot_mask = old_of_new >= 0
    new_of_old = np.zeros(N, np.int64)
    new_of_old[old_of_new[slot_mask]] = np.nonzero(slot_mask)[0]
    src = new_of_old[src]
    dst = new_of_old[dst]
    x_pad = np.zeros((NTOT, P), np.float32)
    x_pad[slot_mask] = x[old_of_new[slot_mask]]
    batch_new = np.zeros(NTOT, np.int64)
    batch_new[slot_mask] = batch[old_of_new[slot_mask]]
    pool_new = np.full(NTOT, -1.0, np.float32)
    pool_new[slot_mask] = batch[old_of_new[slot_mask]]

    blk = dst // P
    is_hi = src >= LO_SPLIT
    order = np.lexsort((is_hi, blk))
    blk_s = blk[order]
    hi_s = is_hi[order]
    src_s = src[order]
    dst_s = dst[order]
    nrm_s = norm[order]
    bstart = np.searchsorted(blk_s, np.arange(NBT))
    bend = np.searchsorted(blk_s, np.arange(NBT), side="right")
    n_lo_b = np.zeros(NBT, np.int64)
    n_hi_b = np.zeros(NBT, np.int64)
    for b in range(NBT):
        seg_hi = hi_s[bstart[b]:bend[b]]
        n_hi_b[b] = seg_hi.sum()
        n_lo_b[b] = len(seg_hi) - n_hi_b[b]
    CL = max(1, math.ceil(n_lo_b.max() / P))
    any_hi = (NTOT > LO_SPLIT) and n_hi_b.max() > 0
    CH = max(1, math.ceil(n_hi_b.max() / P)) if any_hi else 0
    CPB = CL + CH

    idx_lo = np.full((NCORES, P, NBC * CL * 8), -1, np.int16)
    idx_hi = np.full((NCORES, P, NBC * CH * 8), -1, np.int16) if CH else None
    dstl = np.full((NCORES, P, NBC * CPB), -1.0, np.float32)
    nrmv = np.zeros((NCORES, P, NBC * CPB), np.float32)
    cnts = np.zeros((NCORES, 1, NBC * 2), np.int32)

    for b in range(NBT):
        k, lb = divmod(b, NBC)
        s0, s1 = bstart[b], bend[b]
        nl = int(n_lo_b[b])
        nh = int(n_hi_b[b])
        # lo side
        fl = np.full(CL * P, -1, np.int64)
        dl = np.full(CL * P, -1.0, np.float32)
        vl = np.zeros(CL * P, np.float32)
        if nl:
            fl[:nl] = src_s[s0:s0 + nl]
            dl[:nl] = dst_s[s0:s0 + nl] - b * P
            vl[:nl] = nrm_s[s0:s0 + nl]
        else:
            fl[0] = 0  # dummy valid idx (norm stays 0)
            nl = 1
        idx_lo[k][:, lb * CL * 8:(lb + 1) * CL * 8] = _wrap16(fl, -1)
        dstl[k][:, lb * CPB:lb * CPB + CL] = dl.reshape(CL, P).T
        nrmv[k][:, lb * CPB:lb * CPB + CL] = vl.reshape(CL, P).T
        cnts[k][0, 2 * lb] = nl
        # hi side
        if CH:
            fh = np.full(CH * P, -1, np.int64)
            dh = np.full(CH * P, -1.0, np.float32)
            vh = np.zeros(CH * P, np.float32)
            if nh:
                fh[:nh] = src_s[s1 - nh:s1] - LO_SPLIT
                dh[:nh] = dst_s[s1 - nh:s1] - b * P
                vh[:nh] = nrm_s[s1 - nh:s1]
            else:
                fh[0] = 0
                nh = 1
            idx_hi[k][:, lb * CH * 8:(lb + 1) * CH * 8] = _wrap16(fh, -1)
            dstl[k][:, lb * CPB + CL:(lb + 1) * CPB] = dh.reshape(CH, P).T
            nrmv[k][:, lb * CPB + CL:(lb + 1) * CPB] = vh.reshape(CH, P).T
            cnts[k][0, 2 * lb + 1] = nh

    # res gather indices (graph id per node) and pooling one-hot scalars
    batch_pad = batch_new
    pool_v = pool_new
    res_groups = []
    off = 0
    while off < NPC:
        g = min(RES_GRP, NPC - off)
        res_groups.append(g)
        off += g
    idx_res = np.zeros((NCORES, P, NPC // 16), np.int16)
    batchv = np.zeros((NCORES, P, NBC), np.float32)
    for k in range(NCORES):
        off = 0
        col = 0
        for g in res_groups:
            seg = batch_pad[k * NPC + off:k * NPC + off + g]
            idx_res[k][:, col:col + g // 16] = _wrap16(seg, 0)
            off += g
            col += g // 16
        batchv[k] = pool_v[k * NPC:(k + 1) * NPC].reshape(NBC, P).T

    counts = np.bincount(batch, minlength=G).astype(np.float32)
    inv_cnt = np.zeros((P, 1), np.float32)
    inv_cnt[:G, 0] = 1.0 / np.maximum(counts, 1.0)

    xT = np.ascontiguousarray(x_pad.T).astype(dt_np)

    gh_f = np.zeros((P, H), np.float32)
    gh_f[:G] = graph_hidden
    iota = np.tile(np.arange(P, dtype=np.float32)[None, :], (P, 1))
    ident = np.eye(P, dtype=np.float32)

    ws_np = np.zeros((L, HK, P, H), np.float32)
    for l in range(L):
        for kk in range(HK):
            ws_np[l, kk] = Ws[l][kk * P:(kk + 1) * P, :]
    wlin_np = np.zeros((HK, P, C), np.float32)
    for kk in range(HK):
        wlin_np[kk] = Wlin[kk * P:(kk + 1) * P, :]

    has_bias = bool(np.any(b0 != 0) or np.any(bs != 0) or np.any(blin != 0))
    bias_np = np.zeros((L + 1, P, H), np.float32)
    bias_np[0] = np.tile(b0[None, :], (P, 1))
    for l in range(L):
        bias_np[l + 1] = np.tile(bs[l][None, :], (P, 1))
    blin_np = np.tile(blin[None, :], (P, 1)).astype(np.float32)

    if os.environ.get("BASSGNN_VERBOSE", "1") == "1":
        print(f"[kernel] CL={CL} CH={CH} CPB={CPB} NBC={NBC} NTOT={NTOT}")
    cfg = dict(N=N, H=H, G=G, L=L, C=C, HK=HK, NBC=NBC, NPC=NPC, NTOT=NTOT,
               CL=CL, CH=CH, CPB=CPB, res_groups=tuple(res_groups),
               has_bias=has_bias)

    in_maps = []
    for k in range(NCORES):
        m = {
            "negdstl": -dstl[k],
            "negnrm": -nrmv[k],
            "xT": xT,
            "w0": W0.astype(dt_np),
            "ws": ws_np.astype(dt_np),
            "wlin": wlin_np.astype(np.float32),
            "gh": gh_f,
            "gh_dt": gh_f.astype(dt_np),
            "iota": iota.astype(dt_np),
            "ident": ident.astype(dt_np),
            "ident32": ident,
            "inv_cnt": inv_cnt,
            "idx_lo": idx_lo[k],
            "dstl": dstl[k],
            "nrmv": nrmv[k],
            "cnts": cnts[k],
            "idx_res": idx_res[k],
            "batchv": batchv[k],
        }
        if CH:
            m["idx_hi"] = idx_hi[k]
        if has_bias:
            m["biasb"] = bias_np
            m["blinb"] = blin_np
        in_maps.append(m)
    return cfg, in_maps


def _build(cfg, dt):
    H, G, L, C, HK = cfg["H"], cfg["G"], cfg["L"], cfg["C"], cfg["HK"]
    NBC, NPC, NTOT = cfg["NBC"], cfg["NPC"], cfg["NTOT"]
    CL, CH, CPB = cfg["CL"], cfg["CH"], cfg["CPB"]
    res_groups = cfg["res_groups"]
    has_bias = cfg["has_bias"]
    NBT = NTOT // P
    f32 = mybir.dt.float32
    AF = mybir.ActivationFunctionType

    nc = bacc.Bacc("TRN2", target_bir_lowering=False, debug=False,
                   num_devices=NCORES, enable_asserts=False, num_swdge_queues=2)

    xT_d = nc.dram_tensor("xT", [P, NTOT], dt, kind="ExternalInput")
    w0_d = nc.dram_tensor("w0", [P, H], dt, kind="ExternalInput")
    ws_d = nc.dram_tensor("ws", [L, HK, P, H], dt, kind="ExternalInput")
    wlin_d = nc.dram_tensor("wlin", [HK, P, C], f32, kind="ExternalInput")
    gh_d = nc.dram_tensor("gh", [P, H], f32, kind="ExternalInput")
    ghdt_d = nc.dram_tensor("gh_dt", [P, H], dt, kind="ExternalInput")
    iota_d = nc.dram_tensor("iota", [P, P], dt, kind="ExternalInput")
    ident_d = nc.dram_tensor("ident", [P, P], dt, kind="ExternalInput")
    ident32_d = nc.dram_tensor("ident32", [P, P], f32, kind="ExternalInput")
    invc_d = nc.dram_tensor("inv_cnt", [P, 1], f32, kind="ExternalInput")
    idxlo_d = nc.dram_tensor("idx_lo", [P, NBC * CL * 8], mybir.dt.int16, kind="ExternalInput")
    idxhi_d = nc.dram_tensor("idx_hi", [P, NBC * CH * 8], mybir.dt.int16, kind="ExternalInput") if CH else None
    dstl_d = nc.dram_tensor("dstl", [P, NBC * CPB], f32, kind="ExternalInput")
    nrmv_d = nc.dram_tensor("nrmv", [P, NBC * CPB], f32, kind="ExternalInput")
    negdstl_d = nc.dram_tensor("negdstl", [P, NBC * CPB], f32, kind="ExternalInput")
    negnrm_d = nc.dram_tensor("negnrm", [P, NBC * CPB], f32, kind="ExternalInput")
    cnts_d = nc.dram_tensor("cnts", [1, NBC * 2], mybir.dt.int32, kind="ExternalInput")
    idxres_d = nc.dram_tensor("idx_res", [P, NPC // 16], mybir.dt.int16, kind="ExternalInput")
    batchv_d = nc.dram_tensor("batchv", [P, NBC], f32, kind="ExternalInput")
    if has_bias:
        biasb_d = nc.dram_tensor("biasb", [L + 1, P, H], f32, kind="ExternalInput")
        blinb_d = nc.dram_tensor("blinb", [P, C], f32, kind="ExternalInput")

    y_out = nc.dram_tensor("y_out", [P, C], f32, kind="ExternalOutput")
    gm_out = nc.dram_tensor("gm_out", [P, H], f32, kind="ExternalOutput")

    with tile.TileContext(nc) as tc:
        with tc.tile_pool(name="const", bufs=1) as cpool, \
             tc.tile_pool(name="dram", bufs=1, space="DRAM") as dram, \
             tc.tile_pool(name="psum", bufs=1, space="PSUM") as pp, \
             tc.tile_pool(name="sacc", bufs=8) as spool, \
             tc.tile_pool(name="work", bufs=1) as wpool, \
             tc.tile_pool(name="gran", bufs=2) as gpool:

            # ---- resident constants / metadata ----
            w0_sb = cpool.tile([P, H], dt)
            nc.sync.dma_start(w0_sb[:], w0_d[:])
            ws_sb = cpool.tile([P, L, HK, H], dt)
            nc.sync.dma_start(ws_sb[:], ws_d.ap().rearrange("l k p h -> p l k h"))
            wlin_sb = cpool.tile([P, HK, C], f32)
            nc.sync.dma_start(wlin_sb[:], wlin_d.ap().rearrange("k p c -> p k c"))
            gh_sb = cpool.tile([P, H], f32)
            nc.sync.dma_start(gh_sb[:], gh_d[:])
            iota_sb = cpool.tile([P, P], dt)
            nc.sync.dma_start(iota_sb[:], iota_d[:])
            ident_sb = cpool.tile([P, P], dt)
            nc.sync.dma_start(ident_sb[:], ident_d[:])
            ident32_sb = cpool.tile([P, P], f32)
            nc.sync.dma_start(ident32_sb[:], ident32_d[:])
            invc_sb = cpool.tile([P, 1], f32)
            nc.sync.dma_start(invc_sb[:], invc_d[:])
            idxlo_sb = cpool.tile([P, NBC * CL * 8], mybir.dt.int16)
            nc.sync.dma_start(idxlo_sb[:], idxlo_d[:])
            if CH:
                idxhi_sb = cpool.tile([P, NBC * CH * 8], mybir.dt.int16)
                nc.sync.dma_start(idxhi_sb[:], idxhi_d[:])
            dstl_sb = cpool.tile([P, NBC * CPB], f32)
            nc.sync.dma_start(dstl_sb[:], dstl_d[:])
            nrmv_sb = cpool.tile([P, NBC * CPB], f32)
            nc.sync.dma_start(nrmv_sb[:], nrmv_d[:])
            negdstl_sb = cpool.tile([P, NBC * CPB], f32)
            nc.sync.dma_start(negdstl_sb[:], negdstl_d[:])
            negnrm_sb = cpool.tile([P, NBC * CPB], f32)
            nc.sync.dma_start(negnrm_sb[:], negnrm_d[:])
            cnts_sb = cpool.tile([P, NBC * 2], mybir.dt.int32)
            nc.sync.dma_start(cnts_sb[0:1, :], cnts_d[:])
            idxres_sb = cpool.tile([P, NPC // 16], mybir.dt.int16)
            nc.sync.dma_start(idxres_sb[:], idxres_d[:])
            batchv_sb = cpool.tile([P, NBC], f32)
            nc.sync.dma_start(batchv_sb[:], batchv_d[:])
            if has_bias:
                biasb_sb = cpool.tile([P, L + 1, H], f32)
                nc.sync.dma_start(biasb_sb[:], biasb_d.ap().rearrange("l p h -> p l h"))
                blinb_sb = cpool.tile([P, C], f32)
                nc.sync.dma_start(blinb_sb[:], blinb_d[:])

            # ---- DRAM state ----
            zt = dram.tile([NTOT, H], dt)            # replicated z table
            zin = dram.tile([NPC, H], dt)            # this core's z slice
            res_dram = dram.tile([NPC, H], dt)       # graph_hidden[batch] rows
            ar_in = dram.tile([P, H], f32)
            ar_out = dram.tile([P, H], f32)

            # ---- persistent SBUF state ----
            h_sb = wpool.tile([P, NBC, H], dt)       # per-block activations
            msgA = wpool.tile([P, CPB, H], dt)
            msgB = wpool.tile([P, CPB, H], dt)
            nc.vector.memset(msgA[:], 0.0)
            nc.vector.memset(msgB[:], 0.0)

            # ---- res rows: gather graph_hidden[batch] into res_dram ----
            off = 0
            col = 0
            for g in res_groups:
                stg = gpool.tile([P, RES_GRP // P, H], dt, tag="resg")
                nc.gpsimd.dma_gather(
                    stg[:, :g // P, :], ghdt_d.ap(),
                    idxres_sb[:, col:col + g // 16],
                    num_idxs=g, num_idxs_reg=g, elem_size=H,
                    single_packet=False)
                nc.sync.dma_start(
                    res_dram[off:off + g, :].rearrange("(g p) h -> p g h", p=P),
                    stg[:, :g // P, :])
                off += g
                col += g // 16

            # ---- layer 0: z0 = x @ W0, computed for ALL blocks (replicated) ----
            for g0 in range(0, NBT, XG):
                gsz = min(XG, NBT - g0)
                xt_g = gpool.tile([P, XG * P], dt, tag="xtg")
                nc.sync.dma_start(xt_g[:, :gsz * P], xT_d[:, g0 * P:(g0 + gsz) * P])
                zg = gpool.tile([P, XG, H], dt, tag="zg0")
                for j in range(gsz):
                    acc = pp.tile([P, H], f32, tag="acc", bufs=3)
                    nc.tensor.matmul(acc[:], xt_g[:, j * P:(j + 1) * P], w0_sb[:],
                                     start=True, stop=True)
                    if j % 2 == 0:
                        nc.scalar.activation(zg[:, j, :], acc[:], AF.Copy)
                    else:
                        nc.vector.tensor_copy(zg[:, j, :], acc[:])
                nc.sync.dma_start(
                    zt[g0 * P:(g0 + gsz) * P, :].rearrange("(g p) h -> p g h", p=P),
                    zg[:, :gsz, :])

            zt_lo = zt[0:min(LO_SPLIT, NTOT), :]
            zt_hi = zt[LO_SPLIT:NTOT, :] if CH else None

            # ---- aggregation rounds ----
            for r in range(L + 1):
                if r > 0:
                    # phase A: z = (h + res) @ Ws[r-1], node-sharded + AllGather

                    for b in range(NBC):
                        if b % ZG == 0:
                            gsz = min(ZG, NBC - b)
                            res_g = gpool.tile([P, ZG, H], dt, tag="resg")
                            nc.sync.dma_start(
                                res_g[:, :gsz, :],
                                res_dram[b * P:(b + gsz) * P, :].rearrange(
                                    "(g p) h -> p g h", p=P))
                            zg2 = gpool.tile([P, ZG, H], dt, tag="zg2")
                        hpr = gpool.tile([P, H], dt, tag="hpr", bufs=3)
                        nc.vector.tensor_tensor(
                            hpr[:], h_sb[:, b, :], res_g[:, b % ZG, :],
                            mybir.AluOpType.add)
                        hprT = gpool.tile([P, HK * P], dt, tag="hprT", bufs=3)
                        for kk in range(HK):
                            pt = pp.tile([P, P], dt, tag="tr", bufs=2)
                            nc.tensor.transpose(
                                pt[:], hpr[:, kk * P:(kk + 1) * P], ident_sb[:])
                            nc.vector.tensor_copy(hprT[:, kk * P:(kk + 1) * P], pt[:])
                        accz = pp.tile([P, H], f32, tag="acc", bufs=3)
                        for kk in range(HK):
                            nc.tensor.matmul(
                                accz[:], hprT[:, kk * P:(kk + 1) * P],
                                ws_sb[:, r - 1, kk, :],
                                start=(kk == 0), stop=(kk == HK - 1))
                        nc.scalar.activation(zg2[:, b % ZG, :], accz[:], AF.Copy)
                        if b % ZG == ZG - 1 or b == NBC - 1:
                            b0g = (b // ZG) * ZG
                            gsz = b - b0g + 1
                            nc.sync.dma_start(
                                zin[b0g * P:(b0g + gsz) * P, :].rearrange(
                                    "(g p) h -> p g h", p=P),
                                zg2[:, :gsz, :])
                    nc.gpsimd.collective_compute(
                        "AllGather", mybir.AluOpType.bypass,
                        replica_groups=[list(range(NCORES))],
                        ins=[zin[:].opt()], outs=[zt[:].opt()])

                # phase B: gather + segment-sum matmuls per owned block
                for b in range(NBC):
                    msg = msgA if b % 2 == 0 else msgB
                    rlo = nc.gpsimd.alloc_register(f"rlo_{r}_{b}")
                    if CH:
                        rhi = nc.gpsimd.alloc_register(f"rhi_{r}_{b}")
                        nc.gpsimd.reg_load([rlo, rhi], cnts_sb[0:1, 2 * b:2 * b + 2])
                    else:
                        nc.gpsimd.reg_load(rlo, cnts_sb[0:1, 2 * b:2 * b + 1])
                    nc.gpsimd.dma_gather(
                        msg[:, 0:CL, :], zt_lo,
                        idxlo_sb[:, b * CL * 8:(b + 1) * CL * 8],
                        num_idxs=CL * P, num_idxs_reg=rlo, elem_size=H,
                        single_packet=False, queue_num=0)
                    if CH:
                        nc.gpsimd.dma_gather(
                            msg[:, CL:CPB, :], zt_hi,
                            idxhi_sb[:, b * CH * 8:(b + 1) * CH * 8],
                            num_idxs=CH * P, num_idxs_reg=rhi, elem_size=H,
                            single_packet=False, queue_num=1)
                    acch = pp.tile([P, H], f32, tag="acc", bufs=3)
                    for c in range(CPB):
                        col = b * CPB + c
                        S = spool.tile([P, P], dt, tag="S")
                        if c % 9 < 5:
                            nc.vector.tensor_scalar(
                                S[:], iota_sb[:],
                                dstl_sb[:, col:col + 1],
                                nrmv_sb[:, col:col + 1],
                                mybir.AluOpType.is_equal, mybir.AluOpType.mult)
                        else:
                            St = spool.tile([P, P], dt, tag="St", bufs=4)
                            nc.scalar.activation(
                                St[:], iota_sb[:], AF.Abs,
                                bias=negdstl_sb[:, col:col + 1])
                            nc.scalar.activation(
                                S[:], St[:], AF.Relu,
                                bias=nrmv_sb[:, col:col + 1],
                                scale=negnrm_sb[:, col:col + 1])
                        nc.tensor.matmul(acch[:], S[:], msg[:, c, :],
                                         start=(c == 0), stop=(c == CPB - 1))
                    if has_bias:
                        tmpb = gpool.tile([P, H], f32, tag="tmpb", bufs=2)
                        nc.vector.tensor_tensor(
                            tmpb[:], acch[:], biasb_sb[:, r, :],
                            mybir.AluOpType.add)
                        nc.scalar.activation(h_sb[:, b, :], tmpb[:],
                                             AF.Relu if r > 0 else AF.Copy)
                    else:
                        nc.scalar.activation(h_sb[:, b, :], acch[:],
                                             AF.Relu if r > 0 else AF.Copy)

            # ---- graph mean pooling ----
            accg = pp.tile([P, H], f32, tag="gm")
            for b in range(NBC):
                Sp = spool.tile([P, P], dt, tag="S")
                nc.vector.tensor_scalar(
                    Sp[:], iota_sb[:], batchv_sb[:, b:b + 1], None,
                    mybir.AluOpType.is_equal)
                nc.tensor.matmul(accg[:], Sp[:], h_sb[:, b, :],
                                 start=(b == 0), stop=(b == NBC - 1))
            gm_part = wpool.tile([P, H], f32)
            nc.vector.tensor_copy(gm_part[:], accg[:])
            nc.sync.dma_start(ar_in[:], gm_part[:])
            nc.gpsimd.collective_compute(
                "AllReduce", mybir.AluOpType.add,
                replica_groups=[list(range(NCORES))],
                ins=[ar_in[:].opt()], outs=[ar_out[:].opt()])
            gm_sum = wpool.tile([P, H], f32)
            nc.sync.dma_start(gm_sum[:], ar_out[:])
            gm_sb = wpool.tile([P, H], f32)
            nc.vector.tensor_scalar(gm_sb[:], gm_sum[:], invc_sb[:, 0:1], None,
                                    mybir.AluOpType.mult)
            nc.vector.tensor_tensor(gm_sb[:], gm_sb[:], gh_sb[:],
                                    mybir.AluOpType.add)
            nc.sync.dma_start(gm_out[:], gm_sb[:])

            # ---- head: y = gm @ Wlin (+ blin) ----
            gmT = wpool.tile([P, HK * P], f32)
            for kk in range(HK):
                pt = pp.tile([P, P], f32, tag="tr", bufs=2)
                nc.tensor.transpose(pt[:], gm_sb[:, kk * P:(kk + 1) * P],
                                    ident32_sb[:])
                nc.vector.tensor_copy(gmT[:, kk * P:(kk + 1) * P], pt[:])
            accy = pp.tile([P, C], f32, tag="y")
            for kk in range(HK):
                nc.tensor.matmul(accy[:], gmT[:, kk * P:(kk + 1) * P],
                                 wlin_sb[:, kk, :],
                                 start=(kk == 0), stop=(kk == HK - 1))
            y_sb = wpool.tile([P, C], f32)
            if has_bias:
                nc.vector.tensor_tensor(y_sb[:], accy[:], blinb_sb[:],
                                        mybir.AluOpType.add)
            else:
                nc.vector.tensor_copy(y_sb[:], accy[:])
            nc.sync.dma_start(y_out[:], y_sb[:])

    nc.finalize()
    return nc


def kernel(x, edge_index, batch, graph_hidden, W0, b0, Ws, bs, Wlin, blin):
    x = np.asarray(x, np.float32)
    edge_index = np.asarray(edge_index)
    batch = np.asarray(batch)
    graph_hidden = np.asarray(graph_hidden, np.float32)
    W0 = np.asarray(W0, np.float32)
    b0 = np.asarray(b0, np.float32)
    Ws = np.asarray(Ws, np.float32)
    bs = np.asarray(bs, np.float32)
    Wlin = np.asarray(Wlin, np.float32)
    blin = np.asarray(blin, np.float32)

    use_bf16 = os.environ.get("BASSGNN_DT", "bf16") == "bf16"
    dt = mybir.dt.bfloat16 if use_bf16 else mybir.dt.float32
    dt_np = mybir.dt.np(dt)

    cfg, in_maps = _preprocess(x, edge_index, batch, graph_hidden,
                               W0, b0, Ws, bs, Wlin, blin, dt_np)
    key = (x.shape, edge_index.shape, cfg["CL"], cfg["CH"], cfg["has_bias"],
           str(dt), hashlib.sha1(np.ascontiguousarray(edge_index).tobytes()).hexdigest())
    if key not in _CACHE:
        _CACHE[key] = _build(cfg, dt)
    nc = _CACHE[key]

    trace = os.environ.get("BASSGNN_TRACE", "0") == "1"
    res = run_bass_kernel_spmd(nc, in_maps, core_ids=list(range(NCORES)),
                               trace=trace)
    LAST["exec_ns"] = res.exec_time_ns
    LAST["results"] = res
    G = cfg["G"]
    y = np.asarray(res.results[0]["y_out"][:G], np.float32)
    gm = np.asarray(res.results[0]["gm_out"][:G], np.float32)
    return (y, gm)
